# revision 1
# baseline (speedup 1.0000x reference)
"""MeshGraphDecoderConcat on 8 TRN2 NeuronCores.

Strategy: partition GRID nodes (scatter destinations) across the 8 cores;
each core owns rows [c*12500, (c+1)*12500) and processes exactly the edges
whose dst lands in its range (balanced to a common padded count). Zero
collectives. Per core:

  edge phase (tiles of 128 edges, mm-groups of 4 tiles):
    - efeat streamed feat-major (host permuted+transposed, f32r)
    - mesh[src] / grid[dst] gathered feat-major via fp16 transpose-gather
      (per-core compacted mesh table so indices fit int16)
    - mm1 (3 chunks) + SiLU -> hT; mm2 token-major per tile (N=256 pad for
      f32r fast path); LayerNorm stats via DVE reduce + ACT square-accum;
      apply via ACT Identity(scale,bias) -> y_ln fp16
    - scatter-sum via one-hot matmul: S[e,w] = (iota == dstcol[e]) built on
      DVE; psum_window[f, w] += y_ln.T @ S.  Fixed non-overlapping 512-row
      windows; tiles never straddle windows (host pads).
  node phase (per completed window = 512 grid rows):
    - h_destT = window copy; node mm1 on [accum | gridT | cnt-rank1] with
      edge-LN gamma/beta folded into host-premultiplied weights
    - SiLU, mm2 token-major, LayerNorm, +grid residual, write out.

kernel(**inputs) -> [100000, 128] float32
"""
import os
import sys
import math
import numpy as np

sys.path.insert(0, "/opt/trn_rl_repo")

from contextlib import ExitStack

from concourse import bacc, bass, mybir, tile  # noqa: E402
from concourse import bass_utils  # noqa: E402

P = 128
D = 128
HID = 128
N_CORES = 8
N_GRID = 100000
WIN = 512
EPS = 1e-5
GRP = 4  # tiles per matmul group (512 edges)

ROWS_PER_CORE = N_GRID // N_CORES          # 12500
N_WIN = (ROWS_PER_CORE + WIN - 1) // WIN   # 25
ROWS_PAD = N_WIN * WIN                     # 12800

f32 = mybir.dt.float32
f32r = mybir.dt.float32r
f16 = mybir.dt.float16
i16 = mybir.dt.int16


def _bf16_round(x):
    u = np.ascontiguousarray(x, dtype=np.float32).view(np.uint32)
    r = (u + 0x7FFF + ((u >> 16) & 1)) & 0xFFFF0000
    return r.astype(np.uint32).view(np.float32)


def _round_f32r(x):
    hi = _bf16_round(x)
    lo = _bf16_round(x - hi)
    return hi + lo


def _wrap_idx(idx_flat):
    """[n] int16 -> [128, n//16] wrapped layout (slot i -> [i%16, i//16]),
    replicated to 128 partitions."""
    n = idx_flat.shape[0]
    assert n % 16 == 0
    w = idx_flat.reshape(n // 16, 16).T.copy()  # [16, n//16]
    return np.tile(w, (8, 1))


def _prep(m2g_efeat, grid_nfeat, mesh_nfeat, eW1, eb1, eW2, eb2, eg, ebt,
          nW1, nb1, nW2, nb2, ng, nbt, src_idx, dst_idx):
    """Host-side sharding/layout prep. Returns (meta, in_maps)."""
    E = m2g_efeat.shape[0]
    core_of = (dst_idx // ROWS_PER_CORE).astype(np.int64)
    np.minimum(core_of, N_CORES - 1, out=core_of)  # guard (exact division here)

    per_core = []
    t_w_all = np.zeros((N_CORES, N_WIN), dtype=np.int64)
    for c in range(N_CORES):
        eids = np.nonzero(core_of == c)[0]
        dloc = (dst_idx[eids] - c * ROWS_PER_CORE).astype(np.int64)
        order = np.argsort(dloc, kind="stable")
        eids = eids[order]
        dloc = dloc[order]
        win = dloc >> 9  # // 512
        cnt_w = np.bincount(win, minlength=N_WIN)
        t_w_all[c] = (cnt_w + P - 1) // P
        per_core.append((eids, dloc, cnt_w))

    T_w = np.maximum(t_w_all.max(axis=0), 1)
    # pad total tile count to a multiple of GRP by extending the last window
    T_tot = int(T_w.sum())
    if T_tot % GRP:
        T_w[-1] += GRP - (T_tot % GRP)
        T_tot = int(T_w.sum())
    E_pad = T_tot * P
    tile_win = np.repeat(np.arange(N_WIN), T_w)  # window id per tile

    # per-core compact mesh tables must share a common row count
    uniq_list = []
    for c in range(N_CORES):
        eids = per_core[c][0]
        uniq = np.unique(src_idx[eids])
        uniq_list.append(uniq)
    U_pad = max(1, max(len(u) for u in uniq_list))
    assert U_pad <= 32767, f"compact mesh table too large for int16: {U_pad}"

    any_eb2 = bool(np.any(eb2 != 0.0))
    any_nb2 = bool(np.any(nb2 != 0.0))
    any_ebt = bool(np.any(ebt != 0.0))
    any_ng = bool(np.any(ng != 1.0))

    meta = dict(T_w=[int(x) for x in T_w], T_tot=T_tot, E_pad=E_pad,
                U_pad=int(U_pad), any_eb2=any_eb2, any_nb2=any_nb2,
                any_ebt=any_ebt, any_ng=any_ng)

    # ---- shared (weight) arrays ----
    W1e = _round_f32r(eW1[0:D, :])                       # [128,128] f32r
    W1s = eW1[D:2 * D, :].astype(np.float16)
    W1d = eW1[2 * D:3 * D, :].astype(np.float16)
    W2pad = np.zeros((HID, 256), np.float32)
    W2pad[:, :D] = eW2
    W2pad = _round_f32r(W2pad)
    eb2pad = np.zeros((1, 256), np.float32)
    eb2pad[0, :D] = eb2
    eb2pad = _round_f32r(eb2pad)
    Wt = _round_f32r(nW1[0:D, :] * eg[:, None])          # diag(eg) @ nW1_c0
    nW1c1 = _round_f32r(nW1[D:2 * D, :])
    nW2pad = np.zeros((HID, 256), np.float32)
    nW2pad[:, :D] = nW2
    nW2pad = _round_f32r(nW2pad)
    nb2pad = np.zeros((1, 256), np.float32)
    nb2pad[0, :D] = nb2
    nb2pad = _round_f32r(nb2pad)
    v_row = _round_f32r((ebt @ nW1[0:D, :]).reshape(1, HID))
    ones_row = np.ones((1, P), np.float32)  # exact in f32r
    eb1_col = eb1.reshape(P, 1).astype(np.float32)
    nb1_col = nb1.reshape(P, 1).astype(np.float32)
    ngb = np.broadcast_to(ng.astype(np.float32), (P, D))
    ng512 = np.tile(ngb, (1, GRP)).copy()                # [128, 512]

    shared = dict(W1e=W1e, W1s=W1s, W1d=W1d, W2pad=W2pad, eb2pad=eb2pad,
                  Wt=Wt, nW1c1=nW1c1, nW2pad=nW2pad, nb2pad=nb2pad,
                  v_row=v_row, ones_row=ones_row, eb1=eb1_col, nb1=nb1_col,
                  ng512=ng512)

    # ---- per-core arrays ----
    in_maps = []
    for c in range(N_CORES):
        eids, dloc, cnt_w = per_core[c]
        # slot assignment: tiles of window w take that window's edges in order
        slots_eid = np.full(E_pad, -1, dtype=np.int64)
        slot_dcol = np.full(E_pad, -1.0, dtype=np.float32)
        pos = 0
        epos = 0
        for w in range(N_WIN):
            k = int(cnt_w[w]) if w < len(cnt_w) else 0
            slots_eid[pos:pos + k] = eids[epos:epos + k]
            slot_dcol[pos:pos + k] = (dloc[epos:epos + k] - w * WIN)
            epos += k
            pos += int(T_w[w]) * P
        valid = slots_eid >= 0
        sl = np.where(valid, slots_eid, 0)

        efT = np.zeros((E_pad, D), np.float32)
        efT[valid] = m2g_efeat[slots_eid[valid]]
        efT = _round_f32r(np.ascontiguousarray(efT.T))    # [128, E_pad]

        uniq = uniq_list[c]
        inv = np.searchsorted(uniq, src_idx[sl]).astype(np.int16)
        inv[~valid] = 0
        mesh_tbl = np.zeros((U_pad, D), np.float16)
        mesh_tbl[:len(uniq)] = mesh_nfeat[uniq].astype(np.float16)

        grid_shard = np.zeros((ROWS_PAD, D), np.float32)
        nrows = ROWS_PER_CORE
        grid_shard[:nrows] = grid_nfeat[c * nrows:(c + 1) * nrows]
        grid_tbl16 = grid_shard.astype(np.float16)

        didx = np.where(valid, (dst_idx[sl] - c * nrows), 0).astype(np.int16)

        dstcol = slot_dcol.reshape(T_tot, P).T.copy()     # [128, T_tot]

        gridT = _round_f32r(np.ascontiguousarray(grid_shard.T))  # [128, 12800]
        grid_res = grid_shard + nbt[None, :].astype(np.float32)  # residual+nbt
        cnt = np.zeros((1, ROWS_PAD), np.float32)
        cnt[0, :nrows] = np.bincount(
            (dst_idx[eids] - c * nrows).astype(np.int64), minlength=nrows
        ).astype(np.float32)
        cnt = _round_f32r(cnt)

        im = dict(efT=efT, mesh_tbl=mesh_tbl, grid_tbl=grid_tbl16,
                  src_ids=_wrap_idx(inv), dst_ids=_wrap_idx(didx),
                  dstcol=dstcol, gridT=gridT, grid_res=grid_res, cnt=cnt)
        im.update(shared)
        in_maps.append(im)
    return meta, in_maps


def _build(meta):
    T_w = meta["T_w"]
    T_tot = meta["T_tot"]
    E_pad = meta["E_pad"]
    U_pad = meta["U_pad"]

    nc = bacc.Bacc(None, target_bir_lowering=False)

    d_efT = nc.dram_tensor("efT", [P, E_pad], f32r, kind="ExternalInput")
    d_mesh = nc.dram_tensor("mesh_tbl", [U_pad, D], f16, kind="ExternalInput")
    d_grid16 = nc.dram_tensor("grid_tbl", [ROWS_PAD, D], f16, kind="ExternalInput")
    d_sidx = nc.dram_tensor("src_ids", [P, T_tot * 8], i16, kind="ExternalInput")
    d_didx = nc.dram_tensor("dst_ids", [P, T_tot * 8], i16, kind="ExternalInput")
    d_dcol = nc.dram_tensor("dstcol", [P, T_tot], f32, kind="ExternalInput")
    d_gridT = nc.dram_tensor("gridT", [P, ROWS_PAD], f32r, kind="ExternalInput")
    d_gres = nc.dram_tensor("grid_res", [ROWS_PAD, D], f32, kind="ExternalInput")
    d_cnt = nc.dram_tensor("cnt", [1, ROWS_PAD], f32r, kind="ExternalInput")
    d_W1e = nc.dram_tensor("W1e", [D, HID], f32r, kind="ExternalInput")
    d_W1s = nc.dram_tensor("W1s", [D, HID], f16, kind="ExternalInput")
    d_W1d = nc.dram_tensor("W1d", [D, HID], f16, kind="ExternalInput")
    d_W2p = nc.dram_tensor("W2pad", [HID, 256], f32r, kind="ExternalInput")
    d_eb2p = nc.dram_tensor("eb2pad", [1, 256], f32r, kind="ExternalInput")
    d_Wt = nc.dram_tensor("Wt", [D, HID], f32r, kind="ExternalInput")
    d_nW1c1 = nc.dram_tensor("nW1c1", [D, HID], f32r, kind="ExternalInput")
    d_nW2p = nc.dram_tensor("nW2pad", [HID, 256], f32r, kind="ExternalInput")
    d_nb2p = nc.dram_tensor("nb2pad", [1, 256], f32r, kind="ExternalInput")
    d_v = nc.dram_tensor("v_row", [1, HID], f32r, kind="ExternalInput")
    d_ones = nc.dram_tensor("ones_row", [1, P], f32r, kind="ExternalInput")
    d_eb1 = nc.dram_tensor("eb1", [P, 1], f32, kind="ExternalInput")
    d_nb1 = nc.dram_tensor("nb1", [P, 1], f32, kind="ExternalInput")
    d_ng512 = nc.dram_tensor("ng512", [P, GRP * D], f32, kind="ExternalInput")
    d_out = nc.dram_tensor("out", [ROWS_PER_CORE, D], f32, kind="ExternalOutput")

    AF = mybir.ActivationFunctionType
    AL = mybir.AluOpType

    tile_win = []
    for w, tw in enumerate(T_w):
        tile_win += [w] * tw
    first_of_win = {}
    last_of_win = {}
    for t, w in enumerate(tile_win):
        if w not in first_of_win:
            first_of_win[w] = t
        last_of_win[w] = t

    i32 = mybir.dt.int32
    MAGIC = 0x5F3759DF

    with tile.TileContext(nc) as tc:
        with ExitStack() as ctx:
            consts = ctx.enter_context(tc.tile_pool(name="consts", bufs=1))
            efp = ctx.enter_context(tc.tile_pool(name="efp", bufs=3))
            gat = ctx.enter_context(tc.tile_pool(name="gat", bufs=3))
            hbuf = ctx.enter_context(tc.tile_pool(name="hbuf", bufs=2))
            ylnp = ctx.enter_context(tc.tile_pool(name="ylnp", bufs=3))
            sp = ctx.enter_context(tc.tile_pool(name="sp", bufs=4))
            stp = ctx.enter_context(tc.tile_pool(name="stp", bufs=2))
            scr = ctx.enter_context(tc.tile_pool(name="scr", bufs=1))
            hdp = ctx.enter_context(tc.tile_pool(name="hdp", bufs=2))
            ndp = ctx.enter_context(tc.tile_pool(name="ndp", bufs=2))
            outp = ctx.enter_context(tc.tile_pool(name="outp", bufs=2))
            ph_pool = ctx.enter_context(tc.tile_pool(name="ph", bufs=2, space="PSUM"))
            py_pool = ctx.enter_context(tc.tile_pool(name="py", bufs=4, space="PSUM"))
            pw_pool = ctx.enter_context(tc.tile_pool(name="pw", bufs=2, space="PSUM"))

            def cload(dram, shape, dt):
                nm = f"c_{dram.name}"
                t = consts.tile(shape, dt, name=nm, tag=nm)
                nc.sync.dma_start(out=t[:], in_=dram[:])
                return t

            t_W1e = cload(d_W1e, [D, HID], f32r)
            t_W1s = cload(d_W1s, [D, HID], f16)
            t_W1d = cload(d_W1d, [D, HID], f16)
            t_W2p = cload(d_W2p, [HID, 256], f32r)
            t_Wt = cload(d_Wt, [D, HID], f32r)
            t_nW1c1 = cload(d_nW1c1, [D, HID], f32r)
            t_nW2p = cload(d_nW2p, [HID, 256], f32r)
            t_eb1 = cload(d_eb1, [P, 1], f32)
            t_nb1 = cload(d_nb1, [P, 1], f32)
            t_cnt = cload(d_cnt, [1, ROWS_PAD], f32r)
            t_sidx = cload(d_sidx, [P, T_tot * 8], i16)
            t_didx = cload(d_didx, [P, T_tot * 8], i16)
            t_dcol = cload(d_dcol, [P, T_tot], f32)
            if meta["any_eb2"]:
                t_eb2p = cload(d_eb2p, [1, 256], f32r)
            if meta["any_nb2"]:
                t_nb2p = cload(d_nb2p, [1, 256], f32r)
            if meta["any_ebt"]:
                t_v = cload(d_v, [1, HID], f32r)
            if meta["any_ng"]:
                t_ng512 = cload(d_ng512, [P, GRP * D], f32)
            if meta["any_eb2"] or meta["any_nb2"]:
                t_ones = cload(d_ones, [1, P], f32r)

            t_iota = consts.tile([P, WIN], f16, name="c_iota", tag="c_iota")
            nc.gpsimd.iota(t_iota[:], pattern=[[1, WIN]], base=0,
                           channel_multiplier=0,
                           allow_small_or_imprecise_dtypes=True)
            t_sqj = scr.tile([P, 256], f32, name="sqjunk", tag="sqjunk")

            inv128 = 1.0 / 128.0

            # ---- LN helpers: paired-group stats batching ----
            def emit_reduces(p_y, st_sumy, st_sumsq, half):
                """Per-group: -sum(y) via one strided DVE reduce, sum(y^2)
                via ACT Square+accum; columns [half*GRP, ...) of stat tiles."""
                c0 = half * GRP
                nc.vector.tensor_reduce(
                    out=st_sumy[:, c0:c0 + GRP],
                    in_=p_y[:].rearrange("p (t f) -> p t f", t=GRP),
                    axis=mybir.AxisListType.X, op=AL.add, negate=True)
                for t in range(GRP):
                    nc.scalar.activation(
                        out=t_sqj[:, 0:D], in_=p_y[:, t * D:t * D + D],
                        func=AF.Square, accum_out=st_sumsq[:, c0 + t:c0 + t + 1])

            def emit_smalls(st_sumy, st_sumsq, st_rstd, st_b, ncols):
                """Batched over ncols stat columns: rstd = rsqrt(var+eps) via
                magic-constant + 2 Newton iterations (no ACT table swap),
                b = mu_neg * rstd."""
                st_mu = stp.tile([P, 2 * GRP], f32, name="st_mu", tag="st_mu")
                st_v = stp.tile([P, 2 * GRP], f32, name="st_v", tag="st_v")
                st_t1 = stp.tile([P, 2 * GRP], f32, name="st_t1", tag="st_t1")
                st_t2 = stp.tile([P, 2 * GRP], f32, name="st_t2", tag="st_t2")
                cs = slice(0, ncols)
                nc.vector.tensor_scalar(out=st_mu[:, cs], in0=st_sumy[:, cs],
                                        scalar1=inv128, scalar2=None, op0=AL.mult)
                # v = sumsq/128 + eps - mu^2
                nc.vector.tensor_scalar(out=st_v[:, cs], in0=st_sumsq[:, cs],
                                        scalar1=inv128, scalar2=EPS,
                                        op0=AL.mult, op1=AL.add)
                nc.vector.tensor_tensor(out=st_t1[:, cs], in0=st_mu[:, cs],
                                        in1=st_mu[:, cs], op=AL.mult)
                nc.vector.tensor_tensor(out=st_v[:, cs], in0=st_v[:, cs],
                                        in1=st_t1[:, cs], op=AL.subtract)
                # r0 = magic - (v_bits >> 1)
                nc.vector.tensor_scalar(
                    out=st_t1[:, cs].bitcast(i32), in0=st_v[:, cs].bitcast(i32),
                    scalar1=1, scalar2=None, op0=AL.arith_shift_right)
                nc.vector.tensor_scalar(
                    out=st_rstd[:, cs].bitcast(i32), in0=st_t1[:, cs].bitcast(i32),
                    scalar1=-1, scalar2=MAGIC, op0=AL.mult, op1=AL.add)
                for _ in range(2):  # newton: r = r*(1.5 - 0.5*v*r*r)
                    nc.vector.tensor_tensor(out=st_t1[:, cs], in0=st_rstd[:, cs],
                                            in1=st_rstd[:, cs], op=AL.mult)
                    nc.vector.tensor_tensor(out=st_t2[:, cs], in0=st_t1[:, cs],
                                            in1=st_v[:, cs], op=AL.mult)
                    nc.vector.tensor_scalar(out=st_t2[:, cs], in0=st_t2[:, cs],
                                            scalar1=-0.5, scalar2=1.5,
                                            op0=AL.mult, op1=AL.add)
                    nc.vector.tensor_tensor(out=st_rstd[:, cs], in0=st_rstd[:, cs],
                                            in1=st_t2[:, cs], op=AL.mult)
                nc.vector.tensor_tensor(out=st_b[:, cs], in0=st_mu[:, cs],
                                        in1=st_rstd[:, cs], op=AL.mult)

            def emit_apply(p_y, out_buf, st_rstd, st_b, st_mu_src, half):
                """Normalize 4 tiles from the PSUM bank into out_buf;
                alternate ACT/DVE for engine balance."""
                c0 = half * GRP
                for t in range(GRP):
                    src_ap = p_y[:, t * D:t * D + D]
                    dst_ap = out_buf[:, t * D:(t + 1) * D]
                    if t % 2 == 0:
                        nc.scalar.activation(
                            out=dst_ap, in_=src_ap, func=AF.Identity,
                            bias=st_b[:, c0 + t:c0 + t + 1],
                            scale=st_rstd[:, c0 + t:c0 + t + 1])
                    else:
                        nc.vector.tensor_scalar(
                            out=dst_ap, in0=src_ap,
                            scalar1=st_mu_src[:, c0 + t:c0 + t + 1],
                            scalar2=st_rstd[:, c0 + t:c0 + t + 1],
                            op0=AL.add, op1=AL.mult)

            # NOTE: apply's DVE variant needs mu_neg (pre-scaled) — keep a
            # dedicated mu_neg-holding tile per pair: reuse st_b? No: DVE form
            # is (y + mu_neg) * rstd, ACT form is y*rstd + b with b=mu_neg*rstd.
            # We keep both st_b (ACT) and st_muneg (DVE).

            def emit_group_front(g):
                """Loads + mm1 + silu + mm2tok for group g. Returns psum banks."""
                t0 = g * GRP
                e0 = t0 * P
                t_ef = efp.tile([P, GRP * P], f32r, name="t_ef", tag="ef")
                nc.sync.dma_start(out=t_ef[:], in_=d_efT[:, e0:e0 + GRP * P])
                t_src = gat.tile([P, 1, GRP * P], f16, name="t_src", tag="src")
                nc.gpsimd.dma_gather(
                    out_ap=t_src[:], in_ap=d_mesh[:],
                    idxs_ap=t_sidx[:16, t0 * 8:(t0 + GRP) * 8],
                    num_idxs=GRP * P, num_idxs_reg=GRP * P,
                    elem_size=D, transpose=True)
                t_dst = gat.tile([P, 1, GRP * P], f16, name="t_dst", tag="dst")
                nc.gpsimd.dma_gather(
                    out_ap=t_dst[:], in_ap=d_grid16[:],
                    idxs_ap=t_didx[:16, t0 * 8:(t0 + GRP) * 8],
                    num_idxs=GRP * P, num_idxs_reg=GRP * P,
                    elem_size=D, transpose=True)
                p_h = ph_pool.tile([P, GRP * P], f32, space="PSUM",
                                   name="p_h", tag="ph")
                nc.tensor.matmul(out=p_h[:], lhsT=t_W1e[:], rhs=t_ef[:],
                                 start=True, stop=False)
                nc.tensor.matmul(out=p_h[:], lhsT=t_W1s[:], rhs=t_src[:, 0, :],
                                 start=False, stop=False)
                nc.tensor.matmul(out=p_h[:], lhsT=t_W1d[:], rhs=t_dst[:, 0, :],
                                 start=False, stop=True)
                t_h = hbuf.tile([P, GRP * P], f32r, name="t_h", tag="h")
                nc.scalar.activation(out=t_h[:], in_=p_h[:], func=AF.Silu,
                                     bias=t_eb1[:], scale=1.0)
                p_y = py_pool.tile([P, WIN], f32, space="PSUM",
                                   name="p_y", tag="py")
                for t in range(GRP):
                    n_out = 256 if t < GRP - 1 else D
                    reg = p_y[:, t * D:t * D + n_out]
                    nc.tensor.matmul(
                        out=reg, lhsT=t_h[:, t * P:(t + 1) * P],
                        rhs=t_W2p[:, :n_out],
                        start=True, stop=not meta["any_eb2"])
                    if meta["any_eb2"]:
                        nc.tensor.matmul(
                            out=reg, lhsT=t_ones[:], rhs=t_eb2p[:, :n_out],
                            start=False, stop=True)
                return p_y

            p_win = [None, None]

            def emit_group_tail(g, p_banks, st_rstd, st_b, st_muneg, half):
                """Apply + S-build + scatter for group g; trigger node groups."""
                t0 = g * GRP
                t_yln = ylnp.tile([P, GRP * P], f16, name="t_yln", tag="yln")
                emit_apply(p_banks, t_yln, st_rstd, st_b, st_muneg, half)
                for t in range(t0, t0 + GRP):
                    w = tile_win[t]
                    if first_of_win[w] == t:
                        p_win[w % 2] = pw_pool.tile([P, WIN], f32,
                                                    space="PSUM", tag="pw",
                                                    name=f"pwin{w}")
                    t_S = sp.tile([P, WIN], f16, name="t_S", tag="S")
                    eng = nc.vector if (t % 2 == 0) else nc.gpsimd
                    eng.tensor_scalar(
                        out=t_S[:], in0=t_iota[:],
                        scalar1=t_dcol[:, t:t + 1], scalar2=None,
                        op0=AL.is_equal)
                    nc.tensor.matmul(
                        out=p_win[w % 2][:],
                        lhsT=t_yln[:, (t - t0) * P:(t - t0 + 1) * P],
                        rhs=t_S[:],
                        start=(first_of_win[w] == t),
                        stop=(last_of_win[w] == t))
                    if last_of_win[w] == t:
                        emit_node_group(w)

            def emit_node_group(w):
                nrow0 = w * WIN
                t_hdT = hdp.tile([P, WIN], f32r, name="t_hdT", tag="hdT")
                nc.vector.tensor_copy(out=t_hdT[:], in_=p_win[w % 2][:])
                t_gT = ndp.tile([P, WIN], f32r, name="t_gT", tag="gT")
                nc.sync.dma_start(out=t_gT[:], in_=d_gridT[:, nrow0:nrow0 + WIN])
                t_gr = ndp.tile([P, GRP, D], f32, name="t_gr", tag="gres")
                nc.sync.dma_start(
                    out=t_gr[:],
                    in_=d_gres[nrow0:nrow0 + WIN, :].rearrange(
                        "(t p) f -> p t f", p=P))
                p_hn = ph_pool.tile([P, WIN], f32, space="PSUM",
                                    name="p_hn", tag="ph")
                nc.tensor.matmul(out=p_hn[:], lhsT=t_Wt[:], rhs=t_hdT[:],
                                 start=True, stop=False)
                last = not meta["any_ebt"]
                nc.tensor.matmul(out=p_hn[:], lhsT=t_nW1c1[:], rhs=t_gT[:],
                                 start=False, stop=last)
                if meta["any_ebt"]:
                    nc.tensor.matmul(out=p_hn[:], lhsT=t_v[:],
                                     rhs=t_cnt[:, nrow0:nrow0 + WIN],
                                     start=False, stop=True)
                t_hn = hbuf.tile([P, WIN], f32r, name="t_hn", tag="h")
                nc.scalar.activation(out=t_hn[:], in_=p_hn[:], func=AF.Silu,
                                     bias=t_nb1[:], scale=1.0)
                p_zn = py_pool.tile([P, WIN], f32, space="PSUM",
                                    name="p_zn", tag="py")
                for t in range(GRP):
                    n_out = 256 if t < GRP - 1 else D
                    reg = p_zn[:, t * D:t * D + n_out]
                    nc.tensor.matmul(
                        out=reg, lhsT=t_hn[:, t * P:(t + 1) * P],
                        rhs=t_nW2p[:, :n_out],
                        start=True, stop=not meta["any_nb2"])
                    if meta["any_nb2"]:
                        nc.tensor.matmul(
                            out=reg, lhsT=t_ones[:], rhs=t_nb2p[:, :n_out],
                            start=False, stop=True)
                n_sumy = stp.tile([P, 2 * GRP], f32, name="n_sumy", tag="n_sumy")
                n_sumsq = stp.tile([P, 2 * GRP], f32, name="n_sumsq", tag="n_sumsq")
                n_rstd = stp.tile([P, 2 * GRP], f32, name="n_rstd", tag="n_rstd")
                n_b = stp.tile([P, 2 * GRP], f32, name="n_b", tag="n_b")
                emit_reduces(p_zn, n_sumy, n_sumsq, 0)
                emit_smalls(n_sumy, n_sumsq, n_rstd, n_b, GRP)
                t_zln = outp.tile([P, GRP * D], f32, name="t_zln", tag="zln")
                st_muneg = stp.tile([P, 2 * GRP], f32, name="n_mun", tag="n_mun")
                nc.vector.tensor_scalar(out=st_muneg[:, 0:GRP],
                                        in0=n_sumy[:, 0:GRP],
                                        scalar1=inv128, scalar2=None, op0=AL.mult)
                emit_apply(p_zn, t_zln, n_rstd, n_b, st_muneg, 0)
                if meta["any_ng"]:
                    nc.vector.tensor_tensor(out=t_zln[:], in0=t_zln[:],
                                            in1=t_ng512[:], op=AL.mult)
                t_o = outp.tile([P, GRP, D], f32, name="t_o", tag="outt")
                nc.vector.tensor_tensor(
                    out=t_o[:], in0=t_zln[:].rearrange("p (t f) -> p t f", t=GRP),
                    in1=t_gr[:], op=AL.add)
                for t in range(GRP):
                    r0 = nrow0 + t * P
                    k = min(P, ROWS_PER_CORE - r0)
                    if k <= 0:
                        break
                    nc.sync.dma_start(out=d_out[r0:r0 + k, :], in_=t_o[:k, t, :])

            # ---------------- main schedule: paired groups ----------------
            n_groups = T_tot // GRP
            g = 0
            while g < n_groups:
                pair = [g] if g + 1 >= n_groups else [g, g + 1]
                st_sumy = stp.tile([P, 2 * GRP], f32, name="st_sumy", tag="st_sumy")
                st_sumsq = stp.tile([P, 2 * GRP], f32, name="st_sumsq", tag="st_sumsq")
                st_rstd = stp.tile([P, 2 * GRP], f32, name="st_rstd", tag="st_rstd")
                st_b = stp.tile([P, 2 * GRP], f32, name="st_b", tag="st_b")
                st_muneg = stp.tile([P, 2 * GRP], f32, name="st_mun", tag="st_mun")
                banks = []
                for half, gg in enumerate(pair):
                    pb = emit_group_front(gg)
                    emit_reduces(pb, st_sumy, st_sumsq, half)
                    banks.append(pb)
                ncols = GRP * len(pair)
                emit_smalls(st_sumy, st_sumsq, st_rstd, st_b, ncols)
                nc.vector.tensor_scalar(out=st_muneg[:, 0:ncols],
                                        in0=st_sumy[:, 0:ncols],
                                        scalar1=inv128, scalar2=None, op0=AL.mult)
                for half, gg in enumerate(pair):
                    emit_group_tail(gg, banks[half], st_rstd, st_b, st_muneg, half)
                g += len(pair)

    nc.compile()
    return nc


def _run_pjrt(nc, in_maps, bench_iters=0):
    """Execute on 8 cores via PJRT shard_map (mirrors bass2jax.run_bass_via_pjrt
    but keeps reusable device buffers so repeat executions can be timed)."""
    import time
    import jax
    from jax.sharding import Mesh, PartitionSpec
    from jax.experimental.shard_map import shard_map
    from concourse import bass2jax, mybir as mb

    bass2jax.install_neuronx_cc_hook()
    n_cores = len(in_maps)
    partition_name = nc.partition_id_tensor.name if nc.partition_id_tensor else None
    in_names, out_names, out_avals, zero_outs = [], [], [], []
    for alloc in nc.m.functions[0].allocations:
        if not isinstance(alloc, mb.MemoryLocationSet):
            continue
        name = alloc.memorylocations[0].name
        if alloc.kind == "ExternalInput":
            if name != partition_name:
                in_names.append(name)
        elif alloc.kind == "ExternalOutput":
            out_names.append(name)
            shape = tuple(alloc.tensor_shape)
            dtype = mb.dt.np(alloc.dtype)
            out_avals.append(jax.core.ShapedArray(shape, dtype))
            zero_outs.append(np.zeros(shape, dtype))
    n_params = len(in_names)
    all_in = list(in_names) + list(out_names)
    if partition_name is not None:
        all_in.append(partition_name)

    def _body(*args):
        operands = list(args)
        if partition_name is not None:
            operands.append(bass2jax.partition_id_tensor())
        outs = bass2jax._bass_exec_p.bind(
            *operands, out_avals=tuple(out_avals), in_names=tuple(all_in),
            out_names=tuple(out_names), lowering_input_output_aliases=(),
            sim_require_finite=False, sim_require_nnan=False, nc=nc)
        return tuple(outs)

    devices = jax.devices()[:n_cores]
    mesh = Mesh(np.asarray(devices), ("core",))
    in_specs = (PartitionSpec("core"),) * (n_params + len(out_names))
    out_specs = (PartitionSpec("core"),) * len(out_names)
    f = jax.jit(shard_map(_body, mesh=mesh, in_specs=in_specs,
                          out_specs=out_specs, check_rep=False))
    concat_in = [np.concatenate([np.asarray(in_maps[c][nm]) for c in range(n_cores)], axis=0)
                 for nm in in_names]
    concat_in += [np.concatenate([z] * n_cores, axis=0) for z in zero_outs]
    sharding = jax.sharding.NamedSharding(mesh, PartitionSpec("core"))
    dev_in = [jax.device_put(a, sharding) for a in concat_in]
    out_arrs = jax.block_until_ready(f(*dev_in))
    times = []
    for _ in range(bench_iters):
        t0 = time.perf_counter()
        r = jax.block_until_ready(f(*dev_in))
        times.append(time.perf_counter() - t0)
        del r
    results = []
    for c in range(n_cores):
        m = {}
        for i, nm in enumerate(out_names):
            full = np.asarray(out_arrs[i])
            rows = full.shape[0] // n_cores
            m[nm] = full[c * rows:(c + 1) * rows]
        results.append(m)
    return results, times


def kernel(**inputs):
    meta, in_maps = _prep(**inputs)
    nc = _build(meta)
    bench = int(os.environ.get("KBENCH_ITERS", "0"))
    results, times = _run_pjrt(nc, in_maps, bench_iters=bench)
    out = np.concatenate([results[c]["out"] for c in range(N_CORES)], axis=0)
    kernel.last_exec_time_ns = int(min(times) * 1e9) if times else None
    kernel.bench_times = times
    return out.astype(np.float32)

